# revision 7
# baseline (speedup 1.0000x reference)
"""Trainium2 Bass kernel for nn_AttentionCIDNN (block-diagonal crowd attention).

Problem: x[8192, 8, 2] -> last timestep -> 3-layer MLP -> h[8192, 64];
128 groups of 64 agents; per group A = h_g @ h_g^T, column-shifted softmax
P = exp(A - m[j]) / (sum_j exp(A - m[j]) + eps); scatter P onto the block
diagonal of an 8192 x 8192 zero matrix.

Sharding: 8 cores, each owns 1024 contiguous agents (16 groups) and writes its
[1024, 8192] row-slab of the output (memory-regime: 32 MB of mostly-zero rows
per core) plus a small "bands" tensor holding the 16 nonzero 64x64 blocks
packed as 8 chunks of [128, 128]. The host pastes bands onto the zero slabs.

Self-contained: hardcodes all shapes; builds the Bass graph once per process.
"""

import numpy as np

import concourse.bass as bass
import concourse.bacc as bacc
import concourse.mybir as mybir
from concourse.tile import TileContext
from concourse.bass_utils import run_bass_kernel_spmd

F32 = mybir.dt.float32

BS = 8192          # total agents
NCORES = 8
AGENTS = BS // NCORES   # 1024 agents per core
CHUNKS = AGENTS // 128  # 8 row-chunks of 128 agents per core
BLK = 64                # agents per attention group
EPS = 1e-7

_NC_CACHE = None
LAST_RESULT = None  # BassKernelResults of the most recent run (for test harness)


def build_nc():
    """Build the single-core Bass graph (identical on all 8 cores)."""
    nc = bacc.Bacc("TRN2", target_bir_lowering=False)

    xT = nc.declare_dram_parameter("xT", [2, AGENTS], F32, isOutput=False)
    w1 = nc.declare_dram_parameter("W1", [2, 32], F32, isOutput=False)
    b1 = nc.declare_dram_parameter("b1", [32, 1], F32, isOutput=False)
    w2 = nc.declare_dram_parameter("W2", [32, 64], F32, isOutput=False)
    b2 = nc.declare_dram_parameter("b2", [64, 1], F32, isOutput=False)
    w3 = nc.declare_dram_parameter("W3", [64, 64], F32, isOutput=False)
    b3 = nc.declare_dram_parameter("b3", [64, 1], F32, isOutput=False)
    out = nc.declare_dram_parameter("out", [AGENTS, BS], F32, isOutput=True)
    bands = nc.declare_dram_parameter("bands", [AGENTS, 128], F32, isOutput=True)

    with TileContext(nc) as tc:
        with (
            tc.tile_pool(name="sb", bufs=1) as sb,
            tc.tile_pool(name="ps", bufs=1, space="PSUM") as ps,
            tc.tile_pool(name="psmlp", bufs=2, space="PSUM") as psmlp,
        ):
            # ---- the memory-bound part: zero the full [1024, 8192] row-slab
            zerot = sb.tile([128, BS], F32)
            nc.vector.memset(zerot, 0.0)
            for c in range(CHUNKS):
                nc.sync.dma_start(out=out[c * 128:(c + 1) * 128, :], in_=zerot[:, :])

            # ---- inputs (ACT-ring HWDGE so they don't queue behind the zeros)
            x_s = sb.tile([2, AGENTS], F32)
            nc.scalar.dma_start(out=x_s, in_=xT[:, :])
            w1_s = sb.tile([2, 32], F32)
            nc.scalar.dma_start(out=w1_s, in_=w1[:, :])
            b1_s = sb.tile([32, 1], F32)
            nc.scalar.dma_start(out=b1_s, in_=b1[:, :])
            w2_s = sb.tile([32, 64], F32)
            nc.scalar.dma_start(out=w2_s, in_=w2[:, :])
            b2_s = sb.tile([64, 1], F32)
            nc.scalar.dma_start(out=b2_s, in_=b2[:, :])
            w3_s = sb.tile([64, 64], F32)
            nc.scalar.dma_start(out=w3_s, in_=w3[:, :])
            b3_s = sb.tile([64, 1], F32)
            nc.scalar.dma_start(out=b3_s, in_=b3[:, :])

            band_ev = sb.tile([64, AGENTS], F32)
            nc.vector.memset(band_ev, 0.0)
            band_od = sb.tile([64, AGENTS], F32)
            nc.vector.memset(band_od, 0.0)

            # ---- MLP (feature-major layout: h_T[d, agent])
            p1 = psmlp.tile([32, AGENTS], F32, tag="mlp")
            for j in range(0, AGENTS, 512):
                nc.tensor.matmul(p1[:, j:j + 512], w1_s, x_s[:, j:j + 512])
            h1 = sb.tile([32, AGENTS], F32)
            nc.scalar.activation(h1, p1, mybir.ActivationFunctionType.Relu,
                                 bias=b1_s, scale=1.0)

            p2 = psmlp.tile([64, AGENTS], F32, tag="mlp")
            for j in range(0, AGENTS, 512):
                nc.tensor.matmul(p2[:, j:j + 512], w2_s, h1[:, j:j + 512])
            h2 = sb.tile([64, AGENTS], F32)
            nc.scalar.activation(h2, p2, mybir.ActivationFunctionType.Relu,
                                 bias=b2_s, scale=1.0)

            p3 = psmlp.tile([64, AGENTS], F32, tag="mlp")
            for j in range(0, AGENTS, 512):
                nc.tensor.matmul(p3[:, j:j + 512], w3_s, h2[:, j:j + 512])
            h3 = sb.tile([64, AGENTS], F32)
            nc.scalar.activation(h3, p3, mybir.ActivationFunctionType.Identity,
                                 bias=b3_s, scale=1.0)

            # ---- block self-attention: all 16 groups side by side on
            # partitions 0:64 (partition_all_reduce and matmul output offsets
            # are only reliable at partition base 0 on HW)
            pA = ps.tile([64, 16 * BLK], F32)
            for b in range(16):
                sl = h3[:, b * BLK:(b + 1) * BLK]
                nc.tensor.matmul(pA[:, b * BLK:(b + 1) * BLK], sl, sl)

            # m[j] per group: A is symmetric, so the row-max of row j equals
            # the column-max of column j -> GPSIMD partition all-reduce (max),
            # which also broadcasts the result back to every partition.
            a_s = sb.tile([64, 16 * BLK], F32)
            nc.vector.tensor_copy(a_s, pA)
            V = sb.tile([64, 16 * BLK], F32)
            nc.gpsimd.partition_all_reduce(
                V, a_s, channels=64, reduce_op=bass.bass_isa.ReduceOp.max)

            d_s = sb.tile([64, 16 * BLK], F32)
            nc.vector.tensor_sub(d_s, pA, V)
            e_s = sb.tile([64, 16 * BLK], F32)
            nc.scalar.activation(e_s, d_s, mybir.ActivationFunctionType.Exp)

            s_sum = sb.tile([64, 16], F32)
            nc.vector.reduce_sum(s_sum,
                                 e_s.rearrange("p (b j) -> p b j", j=BLK),
                                 axis=mybir.AxisListType.X)
            nc.vector.tensor_scalar_add(s_sum, s_sum, EPS)
            rinv = sb.tile([64, 16], F32)
            nc.vector.reciprocal(rinv, s_sum)

            # P = e * (1/(sum+eps)) written into the band staircase layout
            for r in range(8):
                be, bo = 2 * r, 2 * r + 1
                nc.vector.tensor_scalar_mul(
                    band_ev[:, r * 128:r * 128 + 64],
                    e_s[:, be * BLK:(be + 1) * BLK],
                    rinv[:, be:be + 1])
                nc.vector.tensor_scalar_mul(
                    band_od[:, r * 128 + 64:r * 128 + 128],
                    e_s[:, bo * BLK:(bo + 1) * BLK],
                    rinv[:, bo:bo + 1])

            for r in range(CHUNKS):
                nc.scalar.dma_start(out=bands[r * 128:r * 128 + 64, :],
                                    in_=band_ev[:, r * 128:(r + 1) * 128])
                nc.scalar.dma_start(out=bands[r * 128 + 64:(r + 1) * 128, :],
                                    in_=band_od[:, r * 128:(r + 1) * 128])

    nc.compile()
    return nc


def _get_nc():
    global _NC_CACHE
    if _NC_CACHE is None:
        _NC_CACHE = build_nc()
    return _NC_CACHE


def kernel(x, W1, b1, W2, b2, W3, b3, sub_batches, **run_kwargs):
    global LAST_RESULT
    x = np.asarray(x)
    xt = np.ascontiguousarray(x[:, -1, :], dtype=np.float32)  # [8192, 2]
    W1 = np.ascontiguousarray(W1, dtype=np.float32)
    W2 = np.ascontiguousarray(W2, dtype=np.float32)
    W3 = np.ascontiguousarray(W3, dtype=np.float32)
    b1c = np.ascontiguousarray(np.asarray(b1, dtype=np.float32).reshape(32, 1))
    b2c = np.ascontiguousarray(np.asarray(b2, dtype=np.float32).reshape(64, 1))
    b3c = np.ascontiguousarray(np.asarray(b3, dtype=np.float32).reshape(64, 1))

    in_maps = []
    for d in range(NCORES):
        in_maps.append({
            "xT": np.ascontiguousarray(xt[d * AGENTS:(d + 1) * AGENTS, :].T),
            "W1": W1, "b1": b1c, "W2": W2, "b2": b2c, "W3": W3, "b3": b3c,
        })

    nc = _get_nc()
    res = run_bass_kernel_spmd(nc, in_maps, core_ids=list(range(NCORES)),
                               **run_kwargs)
    LAST_RESULT = res

    full = np.vstack([np.asarray(res.results[d]["out"]) for d in range(NCORES)])
    all_bands = [np.asarray(res.results[d]["bands"]) for d in range(NCORES)]
    for g in range(NCORES * CHUNKS):        # 64 global 128-row chunks
        d, r = divmod(g, CHUNKS)
        full[g * 128:(g + 1) * 128, g * 128:(g + 1) * 128] = \
            all_bands[d][r * 128:(r + 1) * 128, :]

    starts = np.asarray(sub_batches)[:, 0]
    canonical = np.array_equal(starts, np.arange(128, dtype=np.int64) * BLK)
    if not canonical:
        # General placement: extract the 64x64 blocks and scatter them at the
        # rows given by sub_batches (faithful to the reference .at[].set).
        scat = np.zeros((BS, BS), dtype=np.float32)
        for n in range(128):
            blk = full[n * BLK:(n + 1) * BLK, n * BLK:(n + 1) * BLK]
            rows = int(starts[n]) + np.arange(BLK)
            scat[np.ix_(rows, rows)] = blk
        full = scat
    return full


# revision 8
# speedup vs baseline: 1.0299x; 1.0299x over previous
"""Trainium2 Bass kernel for nn_AttentionCIDNN (block-diagonal crowd attention).

Problem: x[8192, 8, 2] -> last timestep -> 3-layer MLP -> h[8192, 64];
128 groups of 64 agents; per group A = h_g @ h_g^T, column-shifted softmax
P = exp(A - m[j]) / (sum_j exp(A - m[j]) + eps); scatter P onto the block
diagonal of an 8192 x 8192 zero matrix.

Sharding: 8 cores, each owns 1024 contiguous agents (16 groups) and writes its
[1024, 8192] row-slab of the output (memory-regime: 32 MB of mostly-zero rows
per core) plus a small "bands" tensor holding the 16 nonzero 64x64 blocks
packed as 8 chunks of [128, 128]. The host pastes bands onto the zero slabs.

Self-contained: hardcodes all shapes; builds the Bass graph once per process.
"""

import numpy as np

import concourse.bass as bass
import concourse.bacc as bacc
import concourse.mybir as mybir
from concourse.tile import TileContext
from concourse.bass_utils import run_bass_kernel_spmd

F32 = mybir.dt.float32

BS = 8192          # total agents
NCORES = 8
AGENTS = BS // NCORES   # 1024 agents per core
CHUNKS = AGENTS // 128  # 8 row-chunks of 128 agents per core
BLK = 64                # agents per attention group
EPS = 1e-7

_NC_CACHE = None
LAST_RESULT = None  # BassKernelResults of the most recent run (for test harness)


def build_nc():
    """Build the single-core Bass graph (identical on all 8 cores)."""
    nc = bacc.Bacc("TRN2", target_bir_lowering=False)

    xT = nc.declare_dram_parameter("xT", [2, AGENTS], F32, isOutput=False)
    w1 = nc.declare_dram_parameter("W1", [2, 32], F32, isOutput=False)
    b1 = nc.declare_dram_parameter("b1", [32, 1], F32, isOutput=False)
    w2 = nc.declare_dram_parameter("W2", [32, 64], F32, isOutput=False)
    b2 = nc.declare_dram_parameter("b2", [64, 1], F32, isOutput=False)
    w3 = nc.declare_dram_parameter("W3", [64, 64], F32, isOutput=False)
    b3 = nc.declare_dram_parameter("b3", [64, 1], F32, isOutput=False)
    out = nc.declare_dram_parameter("out", [AGENTS, BS], F32, isOutput=True)
    bands = nc.declare_dram_parameter("bands", [AGENTS, 128], F32, isOutput=True)

    with TileContext(nc) as tc:
        with (
            tc.tile_pool(name="sb", bufs=1) as sb,
            tc.tile_pool(name="ps", bufs=1, space="PSUM") as ps,
            tc.tile_pool(name="psmlp", bufs=2, space="PSUM") as psmlp,
        ):
            # ---- the memory-bound part: zero the full [1024, 8192] row-slab.
            # Two 16 MB DMAs (chunks 0-3 / 4-7), each re-reading the same
            # [128, 8192] SBUF zero tile 4x via a 0-stride repeat dim, so the
            # zero writes occupy only two DMA-completion lanes and can never
            # queue behind compute-gated DMAs. Memset split across DVE+GPSIMD.
            zerot = sb.tile([128, BS], F32)
            nc.vector.memset(zerot[:, 0:BS // 2], 0.0)
            nc.gpsimd.memset(zerot[:, BS // 2:BS], 0.0)
            outv = out[:, :].rearrange("(c p) n -> p c n", p=128)  # [128, 8, 8192]
            zrep = bass.AP(tensor=zerot.tensor, offset=zerot.offset,
                           ap=[list(zerot.ap[0]), [0, CHUNKS // 2],
                               list(zerot.ap[1])])
            nc.sync.dma_start(out=outv[:, 0:CHUNKS // 2, :], in_=zrep)
            nc.sync.dma_start(out=outv[:, CHUNKS // 2:CHUNKS, :], in_=zrep)

            # ---- inputs (ACT-ring HWDGE so they don't queue behind the zeros)
            x_s = sb.tile([2, AGENTS], F32)
            nc.scalar.dma_start(out=x_s, in_=xT[:, :])
            w1_s = sb.tile([2, 32], F32)
            nc.scalar.dma_start(out=w1_s, in_=w1[:, :])
            b1_s = sb.tile([32, 1], F32)
            nc.scalar.dma_start(out=b1_s, in_=b1[:, :])
            w2_s = sb.tile([32, 64], F32)
            nc.scalar.dma_start(out=w2_s, in_=w2[:, :])
            b2_s = sb.tile([64, 1], F32)
            nc.scalar.dma_start(out=b2_s, in_=b2[:, :])
            w3_s = sb.tile([64, 64], F32)
            nc.scalar.dma_start(out=w3_s, in_=w3[:, :])
            b3_s = sb.tile([64, 1], F32)
            nc.scalar.dma_start(out=b3_s, in_=b3[:, :])

            band_ev = sb.tile([64, AGENTS], F32)
            nc.vector.memset(band_ev, 0.0)
            band_od = sb.tile([64, AGENTS], F32)
            nc.vector.memset(band_od, 0.0)

            # ---- MLP (feature-major layout: h_T[d, agent])
            p1 = psmlp.tile([32, AGENTS], F32, tag="mlp")
            for j in range(0, AGENTS, 512):
                nc.tensor.matmul(p1[:, j:j + 512], w1_s, x_s[:, j:j + 512])
            h1 = sb.tile([32, AGENTS], F32)
            nc.scalar.activation(h1, p1, mybir.ActivationFunctionType.Relu,
                                 bias=b1_s, scale=1.0)

            p2 = psmlp.tile([64, AGENTS], F32, tag="mlp")
            for j in range(0, AGENTS, 512):
                nc.tensor.matmul(p2[:, j:j + 512], w2_s, h1[:, j:j + 512])
            h2 = sb.tile([64, AGENTS], F32)
            nc.scalar.activation(h2, p2, mybir.ActivationFunctionType.Relu,
                                 bias=b2_s, scale=1.0)

            p3 = psmlp.tile([64, AGENTS], F32, tag="mlp")
            for j in range(0, AGENTS, 512):
                nc.tensor.matmul(p3[:, j:j + 512], w3_s, h2[:, j:j + 512])
            h3 = sb.tile([64, AGENTS], F32)
            nc.scalar.activation(h3, p3, mybir.ActivationFunctionType.Identity,
                                 bias=b3_s, scale=1.0)

            # ---- block self-attention: all 16 groups side by side on
            # partitions 0:64 (partition_all_reduce and matmul output offsets
            # are only reliable at partition base 0 on HW)
            pA = ps.tile([64, 16 * BLK], F32)
            for b in range(16):
                sl = h3[:, b * BLK:(b + 1) * BLK]
                nc.tensor.matmul(pA[:, b * BLK:(b + 1) * BLK], sl, sl)

            # m[j] per group: A is symmetric, so the row-max of row j equals
            # the column-max of column j -> GPSIMD partition all-reduce (max),
            # which also broadcasts the result back to every partition.
            a_s = sb.tile([64, 16 * BLK], F32)
            nc.vector.tensor_copy(a_s, pA)
            V = sb.tile([64, 16 * BLK], F32)
            nc.gpsimd.partition_all_reduce(
                V, a_s, channels=64, reduce_op=bass.bass_isa.ReduceOp.max)

            d_s = sb.tile([64, 16 * BLK], F32)
            nc.vector.tensor_sub(d_s, pA, V)
            e_s = sb.tile([64, 16 * BLK], F32)
            nc.scalar.activation(e_s, d_s, mybir.ActivationFunctionType.Exp)

            s_sum = sb.tile([64, 16], F32)
            nc.vector.reduce_sum(s_sum,
                                 e_s.rearrange("p (b j) -> p b j", j=BLK),
                                 axis=mybir.AxisListType.X)
            nc.vector.tensor_scalar_add(s_sum, s_sum, EPS)
            rinv = sb.tile([64, 16], F32)
            nc.vector.reciprocal(rinv, s_sum)

            # P = e * (1/(sum+eps)) written into the band staircase layout
            for r in range(8):
                be, bo = 2 * r, 2 * r + 1
                nc.vector.tensor_scalar_mul(
                    band_ev[:, r * 128:r * 128 + 64],
                    e_s[:, be * BLK:(be + 1) * BLK],
                    rinv[:, be:be + 1])
                nc.vector.tensor_scalar_mul(
                    band_od[:, r * 128 + 64:r * 128 + 128],
                    e_s[:, bo * BLK:(bo + 1) * BLK],
                    rinv[:, bo:bo + 1])

            for r in range(CHUNKS):
                nc.scalar.dma_start(out=bands[r * 128:r * 128 + 64, :],
                                    in_=band_ev[:, r * 128:(r + 1) * 128])
                nc.scalar.dma_start(out=bands[r * 128 + 64:(r + 1) * 128, :],
                                    in_=band_od[:, r * 128:(r + 1) * 128])

    nc.compile()
    return nc


def _get_nc():
    global _NC_CACHE
    if _NC_CACHE is None:
        _NC_CACHE = build_nc()
    return _NC_CACHE


def kernel(x, W1, b1, W2, b2, W3, b3, sub_batches, **run_kwargs):
    global LAST_RESULT
    x = np.asarray(x)
    xt = np.ascontiguousarray(x[:, -1, :], dtype=np.float32)  # [8192, 2]
    W1 = np.ascontiguousarray(W1, dtype=np.float32)
    W2 = np.ascontiguousarray(W2, dtype=np.float32)
    W3 = np.ascontiguousarray(W3, dtype=np.float32)
    b1c = np.ascontiguousarray(np.asarray(b1, dtype=np.float32).reshape(32, 1))
    b2c = np.ascontiguousarray(np.asarray(b2, dtype=np.float32).reshape(64, 1))
    b3c = np.ascontiguousarray(np.asarray(b3, dtype=np.float32).reshape(64, 1))

    in_maps = []
    for d in range(NCORES):
        in_maps.append({
            "xT": np.ascontiguousarray(xt[d * AGENTS:(d + 1) * AGENTS, :].T),
            "W1": W1, "b1": b1c, "W2": W2, "b2": b2c, "W3": W3, "b3": b3c,
        })

    nc = _get_nc()
    res = run_bass_kernel_spmd(nc, in_maps, core_ids=list(range(NCORES)),
                               **run_kwargs)
    LAST_RESULT = res

    full = np.vstack([np.asarray(res.results[d]["out"]) for d in range(NCORES)])
    all_bands = [np.asarray(res.results[d]["bands"]) for d in range(NCORES)]
    for g in range(NCORES * CHUNKS):        # 64 global 128-row chunks
        d, r = divmod(g, CHUNKS)
        full[g * 128:(g + 1) * 128, g * 128:(g + 1) * 128] = \
            all_bands[d][r * 128:(r + 1) * 128, :]

    starts = np.asarray(sub_batches)[:, 0]
    canonical = np.array_equal(starts, np.arange(128, dtype=np.int64) * BLK)
    if not canonical:
        # General placement: extract the 64x64 blocks and scatter them at the
        # rows given by sub_batches (faithful to the reference .at[].set).
        scat = np.zeros((BS, BS), dtype=np.float32)
        for n in range(128):
            blk = full[n * BLK:(n + 1) * BLK, n * BLK:(n + 1) * BLK]
            rows = int(starts[n]) + np.arange(BLK)
            scat[np.ix_(rows, rows)] = blk
        full = scat
    return full


# revision 10
# speedup vs baseline: 1.1841x; 1.1497x over previous
"""Trainium2 Bass kernel for nn_AttentionCIDNN (block-diagonal crowd attention).

Problem: x[8192, 8, 2] -> last timestep -> 3-layer MLP -> h[8192, 64];
128 groups of 64 agents; per group A = h_g @ h_g^T, column-shifted softmax
P = exp(A - m[j]) / (sum_j exp(A - m[j]) + eps); scatter P onto the block
diagonal of an 8192 x 8192 zero matrix.

Sharding: 8 cores, each owns 1024 contiguous agents (16 groups) and writes its
[1024, 8192] row-slab of the output (memory-regime: 32 MB of mostly-zero rows
per core) plus a small "bands" tensor holding the 16 nonzero 64x64 blocks
packed as 8 chunks of [128, 128]. The host pastes bands onto the zero slabs.

Only 4 DMAs per core (1 packed input, 2 giant zero writes, 1 band write) so no
DMA ever waits on a reused completion-semaphore lane.

Self-contained: hardcodes all shapes; builds the Bass graph once per process.
"""

import numpy as np

import concourse.bass as bass
import concourse.bacc as bacc
import concourse.mybir as mybir
from concourse.tile import TileContext
from concourse.bass_utils import run_bass_kernel_spmd

F32 = mybir.dt.float32

BS = 8192          # total agents
NCORES = 8
AGENTS = BS // NCORES   # 1024 agents per core
CHUNKS = AGENTS // 128  # 8 row-chunks of 128 agents per core
BLK = 64                # agents per attention group
EPS = 1e-7

# packed input layout: one [64, 1187] f32 blob per core
#   [0:2,    0:1024]  xT      (last-timestep positions, transposed)
#   [0:2, 1024:1056]  W1
#   [0:32,1056:1057]  b1
#   [0:32,1057:1121]  W2
#   [0:64,1121:1122]  b2
#   [0:64,1122:1186]  W3
#   [0:64,1186:1187]  b3
PACK_COLS = 1187

_NC_CACHE = None
LAST_RESULT = None  # BassKernelResults of the most recent run (for test harness)


def build_nc():
    """Build the single-core Bass graph (identical on all 8 cores)."""
    nc = bacc.Bacc("TRN2", target_bir_lowering=False)

    packed = nc.declare_dram_parameter("packed", [64, PACK_COLS], F32,
                                       isOutput=False)
    out = nc.declare_dram_parameter("out", [AGENTS, BS], F32, isOutput=True)
    bands = nc.declare_dram_parameter("bands", [AGENTS, 128], F32, isOutput=True)

    with TileContext(nc) as tc:
        with (
            tc.tile_pool(name="sb", bufs=1) as sb,
            tc.tile_pool(name="ps", bufs=1, space="PSUM") as ps,
            tc.tile_pool(name="psmlp", bufs=2, space="PSUM") as psmlp,
        ):
            # ---- the memory-bound part: zero the full [1024, 8192] row-slab.
            # Two 16 MB DMAs (chunks 0-3 / 4-7), each re-reading the same
            # [128, 8192] SBUF zero tile 4x via a 0-stride repeat dim.
            # Memset split across DVE+GPSIMD to halve time-to-first-DMA.
            zerot = sb.tile([128, BS], F32)
            nc.vector.memset(zerot[:, 0:BS // 2], 0.0)
            nc.gpsimd.memset(zerot[:, BS // 2:BS], 0.0)
            outv = out[:, :].rearrange("(c p) n -> p c n", p=128)  # [128, 8, 8192]
            zrep = bass.AP(tensor=zerot.tensor, offset=zerot.offset,
                           ap=[list(zerot.ap[0]), [0, CHUNKS // 2],
                               list(zerot.ap[1])])
            nc.sync.dma_start(out=outv[:, 0:CHUNKS // 2, :], in_=zrep)
            nc.sync.dma_start(out=outv[:, CHUNKS // 2:CHUNKS, :], in_=zrep)

            # ---- single packed-input DMA (ACT-ring HWDGE, separate from the
            # zero writes on the SP ring)
            packed_s = sb.tile([64, PACK_COLS], F32)
            nc.scalar.dma_start(out=packed_s, in_=packed[:, :])
            x_s = packed_s[0:2, 0:1024]
            w1_s = packed_s[0:2, 1024:1056]
            b1_s = packed_s[0:32, 1056:1057]
            w2_s = packed_s[0:32, 1057:1121]
            b2_s = packed_s[0:64, 1121:1122]
            w3_s = packed_s[0:64, 1122:1186]
            b3_s = packed_s[0:64, 1186:1187]

            # band staircase: [64, 2048]; even groups in cols 0:1024, odd
            # groups in cols 1024:2048, laid out per 128-col output chunk
            band = sb.tile([64, 2 * AGENTS], F32)
            nc.vector.memset(band, 0.0)

            # ---- MLP (feature-major layout: h_T[d, agent])
            p1 = psmlp.tile([32, AGENTS], F32, tag="mlp")
            for j in range(0, AGENTS, 512):
                nc.tensor.matmul(p1[:, j:j + 512], w1_s, x_s[:, j:j + 512])
            h1 = sb.tile([32, AGENTS], F32)
            nc.scalar.activation(h1, p1, mybir.ActivationFunctionType.Relu,
                                 bias=b1_s, scale=1.0)

            p2 = psmlp.tile([64, AGENTS], F32, tag="mlp")
            for j in range(0, AGENTS, 512):
                nc.tensor.matmul(p2[:, j:j + 512], w2_s, h1[:, j:j + 512])
            h2 = sb.tile([64, AGENTS], F32)
            nc.scalar.activation(h2, p2, mybir.ActivationFunctionType.Relu,
                                 bias=b2_s, scale=1.0)

            p3 = psmlp.tile([64, AGENTS], F32, tag="mlp")
            for j in range(0, AGENTS, 512):
                nc.tensor.matmul(p3[:, j:j + 512], w3_s, h2[:, j:j + 512])
            h3 = sb.tile([64, AGENTS], F32)
            nc.scalar.activation(h3, p3, mybir.ActivationFunctionType.Identity,
                                 bias=b3_s, scale=1.0)

            # ---- block self-attention: all 16 groups side by side on
            # partitions 0:64 (partition_all_reduce and matmul output offsets
            # are only reliable at partition base 0 on HW)
            pA = ps.tile([64, 16 * BLK], F32)
            for b in range(16):
                sl = h3[:, b * BLK:(b + 1) * BLK]
                nc.tensor.matmul(pA[:, b * BLK:(b + 1) * BLK], sl, sl)

            # m[j] per group: A is symmetric, so the row-max of row j equals
            # the column-max of column j -> GPSIMD partition all-reduce (max),
            # which also broadcasts the result back to every partition.
            a_s = sb.tile([64, 16 * BLK], F32)
            nc.vector.tensor_copy(a_s, pA)
            V = sb.tile([64, 16 * BLK], F32)
            nc.gpsimd.partition_all_reduce(
                V, a_s, channels=64, reduce_op=bass.bass_isa.ReduceOp.max)

            d_s = sb.tile([64, 16 * BLK], F32)
            nc.vector.tensor_sub(d_s, pA, V)
            e_s = sb.tile([64, 16 * BLK], F32)
            nc.scalar.activation(e_s, d_s, mybir.ActivationFunctionType.Exp)

            s_sum = sb.tile([64, 16], F32)
            nc.vector.reduce_sum(s_sum,
                                 e_s.rearrange("p (b j) -> p b j", j=BLK),
                                 axis=mybir.AxisListType.X)
            nc.vector.tensor_scalar_add(s_sum, s_sum, EPS)
            rinv = sb.tile([64, 16], F32)
            nc.vector.reciprocal(rinv, s_sum)

            # P = e * (1/(sum+eps)) written into the band staircase layout:
            # band chunk k = group index b; even b at cols k*128+0:64, odd b
            # at cols k*128+64:128 (matching output rows b*64..(b+1)*64).
            for b in range(16):
                off = b * 128 + (64 if b % 2 else 0)
                nc.vector.tensor_scalar_mul(
                    band[:, off:off + 64],
                    e_s[:, b * BLK:(b + 1) * BLK],
                    rinv[:, b:b + 1])

            # one DMA for all 16 blocks: bands row = b*64 + p, col = c
            bands_v = bands[:, :].rearrange("(k p) c -> p k c", p=64)
            band_v = band[:, :].rearrange("p (k c) -> p k c", c=128)
            nc.scalar.dma_start(out=bands_v, in_=band_v)

    nc.compile()
    return nc


def _get_nc():
    global _NC_CACHE
    if _NC_CACHE is None:
        _NC_CACHE = build_nc()
    return _NC_CACHE


def pack_inputs(xt_core, W1, b1, W2, b2, W3, b3):
    p = np.zeros((64, PACK_COLS), dtype=np.float32)
    p[0:2, 0:1024] = xt_core.T
    p[0:2, 1024:1056] = W1
    p[0:32, 1056:1057] = b1.reshape(32, 1)
    p[0:32, 1057:1121] = W2
    p[0:64, 1121:1122] = b2.reshape(64, 1)
    p[0:64, 1122:1186] = W3
    p[0:64, 1186:1187] = b3.reshape(64, 1)
    return p


def kernel(x, W1, b1, W2, b2, W3, b3, sub_batches, **run_kwargs):
    global LAST_RESULT
    x = np.asarray(x)
    xt = np.ascontiguousarray(x[:, -1, :], dtype=np.float32)  # [8192, 2]
    W1 = np.asarray(W1, dtype=np.float32)
    W2 = np.asarray(W2, dtype=np.float32)
    W3 = np.asarray(W3, dtype=np.float32)
    b1 = np.asarray(b1, dtype=np.float32)
    b2 = np.asarray(b2, dtype=np.float32)
    b3 = np.asarray(b3, dtype=np.float32)

    in_maps = []
    for d in range(NCORES):
        in_maps.append({"packed": pack_inputs(
            xt[d * AGENTS:(d + 1) * AGENTS, :], W1, b1, W2, b2, W3, b3)})

    nc = _get_nc()
    res = run_bass_kernel_spmd(nc, in_maps, core_ids=list(range(NCORES)),
                               **run_kwargs)
    LAST_RESULT = res

    full = np.vstack([np.asarray(res.results[d]["out"]) for d in range(NCORES)])
    all_bands = [np.asarray(res.results[d]["bands"]) for d in range(NCORES)]
    for g in range(NCORES * CHUNKS):        # 64 global 128-row chunks
        d, r = divmod(g, CHUNKS)
        full[g * 128:(g + 1) * 128, g * 128:(g + 1) * 128] = \
            all_bands[d][r * 128:(r + 1) * 128, :]

    starts = np.asarray(sub_batches)[:, 0]
    canonical = np.array_equal(starts, np.arange(128, dtype=np.int64) * BLK)
    if not canonical:
        # General placement: extract the 64x64 blocks and scatter them at the
        # rows given by sub_batches (faithful to the reference .at[].set).
        scat = np.zeros((BS, BS), dtype=np.float32)
        for n in range(128):
            blk = full[n * BLK:(n + 1) * BLK, n * BLK:(n + 1) * BLK]
            rows = int(starts[n]) + np.arange(BLK)
            scat[np.ix_(rows, rows)] = blk
        full = scat
    return full
